# revision 6
# baseline (speedup 1.0000x reference)
# Trainium2 Bass kernel for nn_CouplingLayer (normalizing-flow coupling layer).
#
# Full inputs in, full outputs out. Sharding: data-parallel over batch,
# B=16 images -> 2 images per core on 8 NeuronCores; small ConvBlock params
# replicated on every core.
#
# Per-core algorithm:
#  - 6-conv residual net (128 hidden ch) on x1, convs as 9-tap shifted
#    matmuls (K=Cin on partitions, 512-pixel tiles) accumulating in PSUM,
#    in float32r (TF32-like) at bf16 speed.
#  - norm scale folded into weights on host; ELU fused into PSUM eviction
#    via the identity  elu(y)+1 = max(y+1, min(exp(y), 1)); activations are
#    stored as h+1 in zero^W one-padded buffers with a host-side bias
#    correction (b_eff = b - sum(W)) for the next layer.
#  - out-conv output channels permuted on host so the 64 matrix channels
#    land in "product layout": psum partition j*8+c = A[c,j].
#  - per-pixel y2 = expm(A) @ x2 via Horner:  u <- x2 + (1/k) A u,
#    where A u is one DVE multiply in product layout + selection-matrix
#    matmuls on the tensor engine. NK=7 terms (||A||_2 <= 0.41).
#  - log_det = trace accumulation via a selection-vector matmul.
import os
import sys

for _p in ("/opt/trn_rl_repo", "/root/.axon_site/_ro/trn_rl_repo"):
    if os.path.isdir(_p) and _p not in sys.path:
        sys.path.insert(0, _p)

import numpy as np

import concourse.bacc as bacc
import concourse.bass as bass
import concourse.mybir as mybir
import concourse.tile as tile
from concourse.bass_utils import run_bass_kernel_spmd

F32 = mybir.dt.float32
F32R = mybir.dt.float32r
AF = mybir.ActivationFunctionType
OP = mybir.AluOpType

N_CORES = 8
B, C_IN, H, W = 16, 16, 64, 64
B_LOC = B // N_CORES            # 2 images per core
HID = 128
C2 = 8                          # x2 channels; per-pixel matrices are 8x8
OUT_CH = 72
NK = 7                          # Horner / Taylor order for expm
HP, WP = H + 2, W + 4           # padded buffer dims (W padded to 68 cols)
NT = H // 8                     # 8 row-tiles of 512 pixels per image


# ---------------------------------------------------------------- host prep
def _np32(a):
    return np.asarray(a, dtype=np.float32)


def prep_host(params):
    """Fold norm scales into conv weights, apply stored=h+1 bias corrections,
    permute out-conv channels, build series constants."""
    P = {}

    def fold(cp):
        w = _np32(cp["w"]) * _np32(cp["s"])[:, None, None, None]
        return w, _np32(cp["b"])

    def lhsT(w):
        # w [cout, cin, kh, kw] -> lhsT [cin, kh*kw, cout]
        return np.ascontiguousarray(np.transpose(w, (1, 2, 3, 0)).reshape(
            w.shape[1], w.shape[2] * w.shape[3], w.shape[0]))

    s_scale = float(_np32(params["s_scale"])[0])
    s_shift = float(_np32(params["s_shift"])[0])
    rescale = float(_np32(params["s_rescale"])[0])
    reshift = float(_np32(params["s_reshift"])[0])

    biases = {}
    w_in, b_in = fold(params["in"])
    P["w_in"] = lhsT(w_in)                       # [8, 9, 128]
    biases["in_e"] = b_in
    biases["in_y1"] = b_in + 1.0

    for i, bp in enumerate(params["blocks"]):
        for cname in ("c1", "c2", "c3"):
            w, b = fold(bp[cname])
            eff = b - w.sum(axis=(1, 2, 3))
            P[f"w_{cname}_{i}"] = lhsT(w)
            biases[f"{cname}_{i}_e"] = eff
            biases[f"{cname}_{i}_y1"] = eff + 1.0

    w_out = _np32(params["out_w"])               # [72, 128, 3, 3]
    b_out = _np32(params["out_b"])
    perm = np.array([p + 8 for p in range(64)] + list(range(8)))
    w_out_p = w_out[perm]
    eff_out = b_out[perm] - w_out_p.sum(axis=(1, 2, 3))
    P["w_out"] = lhsT(w_out_p)                   # [128, 9, 72]
    bias_out = np.zeros((128,), np.float32)
    bias_out[0:64] = s_scale * eff_out[0:64] + s_shift
    bias_out[64:72] = eff_out[64:72]
    biases["out"] = bias_out
    biases["neg1"] = np.full((128,), -1.0, np.float32)

    # pack all bias vectors into one [128, n] tensor (column per name)
    P["bias_names"] = list(biases.keys())
    pack = np.zeros((128, len(biases)), np.float32)
    for j, n in enumerate(P["bias_names"]):
        v = biases[n]
        pack[0:v.shape[0], j] = v
    P["biases"] = pack

    q = np.arange(64)
    mks = []
    for k in range(2, NK + 1):
        M = np.zeros((64, 64), np.float32)
        for pp in range(64):
            M[(q % 8) == (pp // 8), pp] = 1.0 / k
        mks.append(M)
    P["mk"] = np.concatenate(mks, axis=1)        # [64, (NK-1)*64], k=2..NK
    R = np.zeros((8, 64), np.float32)
    for pp in range(64):
        R[pp // 8, pp] = 1.0
    P["r"] = R
    S1 = np.zeros((64, 8), np.float32)
    S1[q, q % 8] = 1.0
    P["s1"] = S1
    P["i8"] = np.eye(8, dtype=np.float32)
    sel = np.zeros((64, 1), np.float32)
    sel[q[(q % 8) == (q // 8)], 0] = rescale
    P["sel"] = sel

    P["scalars"] = dict(s_scale=s_scale, s_shift=s_shift,
                        rescale=rescale, reshift=reshift)
    return P


# ------------------------------------------------------------ device program
def build_program(P):
    nc = bacc.Bacc("TRN2", target_bir_lowering=False, debug=False,
                   num_devices=N_CORES)
    sc = P["scalars"]

    x_d = nc.dram_tensor("x", [B_LOC, C_IN, H, W], F32, kind="ExternalInput")
    wnames = ["w_in", "w_c1_0", "w_c2_0", "w_c3_0",
              "w_c1_1", "w_c2_1", "w_c3_1", "w_out", "mk", "r", "s1", "i8", "sel"]
    wd = {n: nc.dram_tensor(n, list(P[n].reshape(P[n].shape[0], -1).shape),
                            F32, kind="ExternalInput") for n in wnames}
    bias_d = nc.dram_tensor("biases", list(P["biases"].shape), F32,
                            kind="ExternalInput")
    y2_d = nc.dram_tensor("y2", [B_LOC, C2, H, W], F32, kind="ExternalOutput")
    ld_d = nc.dram_tensor("ld", [1, B_LOC], F32, kind="ExternalOutput")

    bcol = {n: j for j, n in enumerate(P["bias_names"])}

    with tile.TileContext(nc) as tc:
        with (
            tc.tile_pool(name="wpool", bufs=1) as wpool,
            tc.tile_pool(name="stage", bufs=1) as stage,
            tc.tile_pool(name="pads", bufs=1) as pads,
            tc.tile_pool(name="tmp", bufs=3) as tmp,
            tc.tile_pool(name="tmp2", bufs=2) as tmp2,
            tc.tile_pool(name="psc", bufs=2, space="PSUM") as psc,
            tc.tile_pool(name="pss", bufs=2, space="PSUM") as pss,
            tc.tile_pool(name="pst", bufs=1, space="PSUM") as pst,
        ):
            # ---- weights: DMA f32 -> SBUF, convert to f32r
            wt = {}
            for n in wnames:
                shp = list(P[n].reshape(P[n].shape[0], -1).shape)
                st = stage.tile(shp, F32, tag="wstage")
                nc.sync.dma_start(out=st, in_=wd[n].ap())
                wt[n] = wpool.tile(shp, F32R, tag=f"wt_{n}", name=f"wt_{n}")
                nc.vector.tensor_copy(out=wt[n], in_=st)
            bias_t = wpool.tile(list(P["biases"].shape), F32, tag="bias")
            nc.sync.dma_start(out=bias_t, in_=bias_d.ap())

            def bias_ap(name, lo, hi):
                return bias_t[lo:hi, bcol[name]:bcol[name] + 1]

            # ---- padded activation buffers
            x1pad = pads.tile([C2, HP * WP], F32R, tag="x1pad")
            nc.vector.memset(x1pad.bitcast(F32), 0.0)
            hbuf = {}
            for n in ("hA", "h1", "h2", "hB"):
                hbuf[n] = pads.tile([HID, HP * WP], F32R, tag=n, name=f"pad_{n}")
                nc.vector.memset(hbuf[n].bitcast(F32), 1.0)

            ld_sb = wpool.tile([1, B_LOC], F32, tag="ld_sb")

            def pv(t, n_ch):
                return t.rearrange("p (h w) -> p h w", h=HP)[0:n_ch]

            def conv(src, n_in, w_t, taps, n_out, ebias, y1bias,
                     dst=None, resid=None, c3bias=None, out_cb=None):
                """One conv layer over 8 row-tiles. If dst: ELU-evict into
                dst's interior. If resid: c3-style residual eviction.
                If out_cb: call out_cb(t_idx, ps_tile) instead of evicting."""
                src_v = pv(src, n_in)
                for t in range(NT):
                    ps_t = psc.tile([128, 512], F32, tag="conv_ps")
                    ti = 0
                    for ty in range(3 if taps == 9 else 1):
                        for tx in range(3 if taps == 9 else 1):
                            if taps == 9:
                                rhs = src_v[:, t * 8 + ty: t * 8 + ty + 8,
                                            tx + 1: tx + 65]
                            else:
                                rhs = src_v[:, t * 8 + 1: t * 8 + 9, 2:66]
                            nc.tensor.matmul(
                                ps_t[0:n_out],
                                wt[w_t][:, ti * n_out:(ti + 1) * n_out],
                                rhs,
                                start=(ti == 0), stop=(ti == taps - 1))
                            ti += 1
                    if out_cb is not None:
                        out_cb(t, ps_t)
                        continue
                    dst_v = pv(dst, n_out)[:, t * 8 + 1: t * 8 + 9, 2:66]
                    if resid is None:
                        e_t = tmp.tile([128, 512], F32, tag="e")
                        nc.scalar.activation(
                            out=e_t[0:n_out], in_=ps_t[0:n_out], func=AF.Exp,
                            bias=bias_ap(ebias, 0, n_out), scale=1.0)
                        y1_t = tmp.tile([128, 512], F32, tag="y1")
                        nc.scalar.activation(
                            out=y1_t[0:n_out], in_=ps_t[0:n_out],
                            func=AF.Identity,
                            bias=bias_ap(y1bias, 0, n_out), scale=1.0)
                    else:
                        res_v = pv(resid, n_out)[:, t * 8 + 1: t * 8 + 9, 2:66]
                        y1_t = tmp.tile([128, 512], F32, tag="y1")
                        nc.vector.scalar_tensor_tensor(
                            out=y1_t[0:n_out].rearrange("p (h w) -> p h w", h=8),
                            in0=ps_t[0:n_out].rearrange("p (h w) -> p h w", h=8),
                            scalar=bias_ap(c3bias, 0, n_out),
                            in1=res_v.bitcast(F32),
                            op0=OP.add, op1=OP.add)
                        e_t = tmp.tile([128, 512], F32, tag="e")
                        nc.scalar.activation(
                            out=e_t[0:n_out], in_=y1_t[0:n_out], func=AF.Exp,
                            bias=bias_ap("neg1", 0, n_out), scale=1.0)
                    nc.vector.scalar_tensor_tensor(
                        out=dst_v,
                        in0=e_t[0:n_out].rearrange("p (h w) -> p h w", h=8),
                        scalar=1.0,
                        in1=y1_t[0:n_out].rearrange("p (h w) -> p h w", h=8),
                        op0=OP.min, op1=OP.max)

            for b in range(B_LOC):
                # ---- load image: x1 into padded interior (f32r), x2 to f32r
                st_x1 = stage.tile([C2, H * W], F32, tag="stage_x1")
                nc.sync.dma_start(
                    out=st_x1,
                    in_=x_d.ap()[b, 0:C2].rearrange("c h w -> c (h w)"))
                st_x2 = stage.tile([C2, H * W], F32, tag="stage_x2")
                nc.sync.dma_start(
                    out=st_x2,
                    in_=x_d.ap()[b, C2:C_IN].rearrange("c h w -> c (h w)"))
                x1_int = pv(x1pad, C2)[:, 1:65, 2:66]
                nc.vector.tensor_copy(
                    out=x1_int,
                    in_=st_x1.rearrange("p (h w) -> p h w", h=H))
                x2r = pads.tile([C2, H * W], F32R, tag="x2r")
                nc.vector.tensor_copy(out=x2r, in_=st_x2)

                # ---- conv net
                conv(x1pad, C2, "w_in", 9, HID, "in_e", "in_y1", dst=hbuf["hA"])
                io = [("hA", "hB"), ("hB", "hA")]
                for i in range(2):
                    sA, sB = io[i]
                    conv(hbuf[sA], HID, f"w_c1_{i}", 9, HID,
                         f"c1_{i}_e", f"c1_{i}_y1", dst=hbuf["h1"])
                    conv(hbuf["h1"], HID, f"w_c2_{i}", 1, HID,
                         f"c2_{i}_e", f"c2_{i}_y1", dst=hbuf["h2"])
                    conv(hbuf["h2"], HID, f"w_c3_{i}", 9, HID,
                         None, None, dst=hbuf[sB], resid=hbuf[sA],
                         c3bias=f"c3_{i}_e")

                # ---- out conv + coupling
                tr_ps = pst.tile([1, 512], F32, tag="tr")

                def couple(t, ps_t):
                    t_t = tmp2.tile([64, 512], F32R, tag="t")
                    nc.scalar.activation(
                        out=t_t, in_=ps_t[0:64], func=AF.Tanh,
                        bias=bias_ap("out", 0, 64), scale=sc["s_scale"])
                    sh_t = tmp2.tile([8, 512], F32, tag="sh")
                    nc.scalar.activation(
                        out=sh_t, in_=ps_t[64:72], func=AF.Identity,
                        bias=bias_ap("out", 64, 72), scale=1.0)
                    # trace accumulation across tiles
                    nc.tensor.matmul(tr_ps, wt["sel"], t_t,
                                     start=(t == 0), stop=(t == NT - 1))
                    # Horner series
                    x2_t = x2r[:, t * 512:(t + 1) * 512]
                    v_ps = pss.tile([64, 512], F32, tag="v")
                    nc.tensor.matmul(v_ps, wt["r"], x2_t, start=True, stop=True)
                    for k in range(NK, 1, -1):
                        p_t = tmp2.tile([64, 512], F32R, tag="p")
                        nc.vector.scalar_tensor_tensor(
                            out=p_t, in0=t_t.bitcast(F32),
                            scalar=sc["rescale"], in1=v_ps,
                            op0=OP.mult, op1=OP.mult)
                        v2 = pss.tile([64, 512], F32, tag="v")
                        nc.tensor.matmul(v2, wt["mk"][:, (k - 2) * 64:(k - 1) * 64],
                                         p_t, start=True, stop=False)
                        nc.tensor.matmul(v2, wt["r"], x2_t,
                                         start=False, stop=True)
                        v_ps = v2
                    p_t = tmp2.tile([64, 512], F32R, tag="p")
                    nc.vector.scalar_tensor_tensor(
                        out=p_t, in0=t_t.bitcast(F32), scalar=sc["rescale"],
                        in1=v_ps, op0=OP.mult, op1=OP.mult)
                    y_ps = pss.tile([8, 512], F32, tag="y")
                    nc.tensor.matmul(y_ps, wt["s1"], p_t, start=True, stop=False)
                    nc.tensor.matmul(y_ps, wt["i8"], x2_t, start=False, stop=True)
                    out_t = tmp.tile([8, 512], F32, tag="out")
                    nc.vector.tensor_add(out=out_t, in0=y_ps, in1=sh_t)
                    nc.sync.dma_start(
                        out=y2_d.ap()[b, :, t * 8:(t + 1) * 8, :],
                        in_=out_t.rearrange("p (h w) -> p h w", h=8))

                conv(hbuf["hA"], HID, "w_out", 9, OUT_CH, None, None,
                     out_cb=couple)
                nc.vector.tensor_reduce(
                    out=ld_sb[:, b:b + 1], in_=tr_ps,
                    axis=mybir.AxisListType.X, op=OP.add)
            nc.sync.dma_start(out=ld_d.ap(), in_=ld_sb)

    nc.finalize()
    return nc


# ------------------------------------------------------------------- driver
_CACHE = {}


def _get_program(P):
    key = "prog"
    if key not in _CACHE:
        _CACHE[key] = build_program(P)
    return _CACHE[key]


def kernel(x, y=None, params=None, _want_trace=False):
    x = _np32(x)
    P = prep_host(params)
    nc = _get_program(P)

    in_maps = []
    for c in range(N_CORES):
        m = {"x": np.ascontiguousarray(x[c * B_LOC:(c + 1) * B_LOC])}
        for n in ["w_in", "w_c1_0", "w_c2_0", "w_c3_0", "w_c1_1", "w_c2_1",
                  "w_c3_1", "w_out", "mk", "r", "s1", "i8", "sel"]:
            m[n] = np.ascontiguousarray(P[n].reshape(P[n].shape[0], -1))
        m["biases"] = P["biases"]
        in_maps.append(m)

    res = run_bass_kernel_spmd(nc, in_maps, core_ids=list(range(N_CORES)),
                               trace=_want_trace)

    y2 = np.concatenate([r["y2"] for r in res.results], axis=0)
    ld = np.concatenate([r["ld"].reshape(-1) for r in res.results], axis=0)
    ld = ld + np.float32(8 * H * W * P["scalars"]["reshift"])
    out = np.concatenate([x[:, 0:C2], y2], axis=1).astype(np.float32)
    if _want_trace:
        return (out, ld.astype(np.float32)), res
    return out, ld.astype(np.float32)


# revision 7
# speedup vs baseline: 1.0638x; 1.0638x over previous
# Trainium2 Bass kernel for nn_CouplingLayer (normalizing-flow coupling layer).
#
# Full inputs in, full outputs out. Sharding: data-parallel over batch,
# B=16 images -> 2 images per core on 8 NeuronCores; small ConvBlock params
# replicated on every core.
#
# Per-core algorithm:
#  - 6-conv residual net (128 hidden ch) on x1, convs as 9-tap shifted
#    matmuls (K=Cin on partitions, 512-pixel tiles) accumulating in PSUM,
#    in float32r (TF32-like) at bf16 speed.
#  - norm scale folded into weights on host; ELU fused into PSUM eviction
#    via the identity  elu(y)+1 = max(y+1, min(exp(y), 1)); activations are
#    stored as h+1 in zero^W one-padded buffers with a host-side bias
#    correction (b_eff = b - sum(W)) for the next layer.
#  - out-conv output channels permuted on host so the 64 matrix channels
#    land in "product layout": psum partition j*8+c = A[c,j].
#  - per-pixel y2 = expm(A) @ x2 via Horner:  u <- x2 + (1/k) A u,
#    where A u is one DVE multiply in product layout + selection-matrix
#    matmuls on the tensor engine. NK=7 terms (||A||_2 <= 0.41).
#  - log_det = trace accumulation via a selection-vector matmul.
import os
import sys

for _p in ("/opt/trn_rl_repo", "/root/.axon_site/_ro/trn_rl_repo"):
    if os.path.isdir(_p) and _p not in sys.path:
        sys.path.insert(0, _p)

import numpy as np

import concourse.bacc as bacc
import concourse.bass as bass
import concourse.mybir as mybir
import concourse.tile as tile
from concourse.bass_utils import run_bass_kernel_spmd

F32 = mybir.dt.float32
F32R = mybir.dt.float32r
FP16 = mybir.dt.float16
BF16 = mybir.dt.bfloat16
MM_DT = {"f32r": F32R, "fp16": FP16, "bf16": BF16}[os.environ.get("KERNEL_MM_DT", "f32r")]
AF = mybir.ActivationFunctionType
OP = mybir.AluOpType

N_CORES = 8
B, C_IN, H, W = 16, 16, 64, 64
B_LOC = B // N_CORES            # 2 images per core
HID = 128
C2 = 8                          # x2 channels; per-pixel matrices are 8x8
OUT_CH = 72
NK = 7                          # Horner / Taylor order for expm
HP, WP = H + 2, W + 4           # padded buffer dims (W padded to 68 cols)
NT = H // 8                     # 8 row-tiles of 512 pixels per image


# ---------------------------------------------------------------- host prep
def _np32(a):
    return np.asarray(a, dtype=np.float32)


def prep_host(params):
    """Fold norm scales into conv weights, apply stored=h+1 bias corrections,
    permute out-conv channels, build series constants."""
    P = {}

    def fold(cp):
        w = _np32(cp["w"]) * _np32(cp["s"])[:, None, None, None]
        return w, _np32(cp["b"])

    def lhsT(w):
        # w [cout, cin, kh, kw] -> lhsT [cin, kh*kw, cout]
        return np.ascontiguousarray(np.transpose(w, (1, 2, 3, 0)).reshape(
            w.shape[1], w.shape[2] * w.shape[3], w.shape[0]))

    s_scale = float(_np32(params["s_scale"])[0])
    s_shift = float(_np32(params["s_shift"])[0])
    rescale = float(_np32(params["s_rescale"])[0])
    reshift = float(_np32(params["s_reshift"])[0])

    biases = {}
    w_in, b_in = fold(params["in"])
    P["w_in"] = lhsT(w_in)                       # [8, 9, 128]
    biases["in_e"] = b_in
    biases["in_y1"] = b_in + 1.0

    for i, bp in enumerate(params["blocks"]):
        for cname in ("c1", "c2", "c3"):
            w, b = fold(bp[cname])
            eff = b - w.sum(axis=(1, 2, 3))
            P[f"w_{cname}_{i}"] = lhsT(w)
            biases[f"{cname}_{i}_e"] = eff
            biases[f"{cname}_{i}_y1"] = eff + 1.0

    w_out = _np32(params["out_w"])               # [72, 128, 3, 3]
    b_out = _np32(params["out_b"])
    perm = np.array([p + 8 for p in range(64)] + list(range(8)))
    w_out_p = w_out[perm]
    eff_out = b_out[perm] - w_out_p.sum(axis=(1, 2, 3))
    P["w_out"] = lhsT(w_out_p)                   # [128, 9, 72]
    bias_out = np.zeros((128,), np.float32)
    bias_out[0:64] = s_scale * eff_out[0:64] + s_shift
    bias_out[64:72] = eff_out[64:72]
    biases["out"] = bias_out
    biases["neg1"] = np.full((128,), -1.0, np.float32)

    # pack all bias vectors into one [128, n] tensor (column per name)
    P["bias_names"] = list(biases.keys())
    pack = np.zeros((128, len(biases)), np.float32)
    for j, n in enumerate(P["bias_names"]):
        v = biases[n]
        pack[0:v.shape[0], j] = v
    P["biases"] = pack

    q = np.arange(64)
    mks = []
    for k in range(2, NK + 1):
        M = np.zeros((64, 64), np.float32)
        for pp in range(64):
            M[(q % 8) == (pp // 8), pp] = 1.0 / k
        mks.append(M)
    P["mk"] = np.concatenate(mks, axis=1)        # [64, (NK-1)*64], k=2..NK
    R = np.zeros((8, 64), np.float32)
    for pp in range(64):
        R[pp // 8, pp] = 1.0
    P["r"] = R
    S1 = np.zeros((64, 8), np.float32)
    S1[q, q % 8] = 1.0
    P["s1"] = S1
    P["i8"] = np.eye(8, dtype=np.float32)
    sel = np.zeros((64, 1), np.float32)
    sel[q[(q % 8) == (q // 8)], 0] = rescale
    P["sel"] = sel

    P["scalars"] = dict(s_scale=s_scale, s_shift=s_shift,
                        rescale=rescale, reshift=reshift)
    return P


# ------------------------------------------------------------ device program
def build_program(P):
    nc = bacc.Bacc("TRN2", target_bir_lowering=False, debug=False,
                   num_devices=N_CORES)
    sc = P["scalars"]

    x_d = nc.dram_tensor("x", [B_LOC, C_IN, H, W], F32, kind="ExternalInput")
    wnames = ["w_in", "w_c1_0", "w_c2_0", "w_c3_0",
              "w_c1_1", "w_c2_1", "w_c3_1", "w_out", "mk", "r", "s1", "i8", "sel"]
    wd = {n: nc.dram_tensor(n, list(P[n].reshape(P[n].shape[0], -1).shape),
                            F32, kind="ExternalInput") for n in wnames}
    bias_d = nc.dram_tensor("biases", list(P["biases"].shape), F32,
                            kind="ExternalInput")
    y2_d = nc.dram_tensor("y2", [B_LOC, C2, H, W], F32, kind="ExternalOutput")
    ld_d = nc.dram_tensor("ld", [1, B_LOC], F32, kind="ExternalOutput")

    bcol = {n: j for j, n in enumerate(P["bias_names"])}

    with tile.TileContext(nc) as tc:
        with (
            tc.tile_pool(name="wpool", bufs=1) as wpool,
            tc.tile_pool(name="stage", bufs=1) as stage,
            tc.tile_pool(name="pads", bufs=1) as pads,
            tc.tile_pool(name="tmp", bufs=3) as tmp,
            tc.tile_pool(name="tmp2", bufs=2) as tmp2,
            tc.tile_pool(name="psc", bufs=2, space="PSUM") as psc,
            tc.tile_pool(name="pss", bufs=2, space="PSUM") as pss,
            tc.tile_pool(name="pst", bufs=1, space="PSUM") as pst,
        ):
            # ---- weights: DMA f32 -> SBUF, convert to f32r
            wt = {}
            for n in wnames:
                shp = list(P[n].reshape(P[n].shape[0], -1).shape)
                st = stage.tile(shp, F32, tag="wstage")
                nc.sync.dma_start(out=st, in_=wd[n].ap())
                wt[n] = wpool.tile(shp, MM_DT, tag=f"wt_{n}", name=f"wt_{n}")
                nc.vector.tensor_copy(out=wt[n], in_=st)
            bias_t = wpool.tile(list(P["biases"].shape), F32, tag="bias")
            nc.sync.dma_start(out=bias_t, in_=bias_d.ap())

            def bias_ap(name, lo, hi):
                return bias_t[lo:hi, bcol[name]:bcol[name] + 1]

            # ---- padded activation buffers
            x1pad = pads.tile([C2, HP * WP], MM_DT, tag="x1pad")
            nc.vector.memset(x1pad.bitcast(F32) if MM_DT == F32R else x1pad, 0.0)
            hbuf = {}
            for n in ("hA", "h1", "h2", "hB"):
                hbuf[n] = pads.tile([HID, HP * WP], MM_DT, tag=n, name=f"pad_{n}")
                nc.vector.memset(hbuf[n].bitcast(F32) if MM_DT == F32R else hbuf[n], 1.0)

            ld_sb = wpool.tile([1, B_LOC], F32, tag="ld_sb")

            def pv(t, n_ch):
                return t.rearrange("p (h w) -> p h w", h=HP)[0:n_ch]

            def conv(src, n_in, w_t, taps, n_out, ebias, y1bias,
                     dst=None, resid=None, c3bias=None, out_cb=None):
                """One conv layer over 8 row-tiles. If dst: ELU-evict into
                dst's interior. If resid: c3-style residual eviction.
                If out_cb: call out_cb(t_idx, ps_tile) instead of evicting."""
                src_v = pv(src, n_in)
                for t in range(NT):
                    ps_t = psc.tile([128, 512], F32, tag="conv_ps")
                    ti = 0
                    for ty in range(3 if taps == 9 else 1):
                        for tx in range(3 if taps == 9 else 1):
                            if taps == 9:
                                rhs = src_v[:, t * 8 + ty: t * 8 + ty + 8,
                                            tx + 1: tx + 65]
                            else:
                                rhs = src_v[:, t * 8 + 1: t * 8 + 9, 2:66]
                            nc.tensor.matmul(
                                ps_t[0:n_out],
                                wt[w_t][:, ti * n_out:(ti + 1) * n_out],
                                rhs,
                                start=(ti == 0), stop=(ti == taps - 1))
                            ti += 1
                    if out_cb is not None:
                        out_cb(t, ps_t)
                        continue
                    dst_v = pv(dst, n_out)[:, t * 8 + 1: t * 8 + 9, 2:66]
                    if resid is None:
                        e_t = tmp.tile([128, 512], F32, tag="e")
                        nc.scalar.activation(
                            out=e_t[0:n_out], in_=ps_t[0:n_out], func=AF.Exp,
                            bias=bias_ap(ebias, 0, n_out), scale=1.0)
                        y1_t = tmp.tile([128, 512], F32, tag="y1")
                        nc.scalar.activation(
                            out=y1_t[0:n_out], in_=ps_t[0:n_out],
                            func=AF.Identity,
                            bias=bias_ap(y1bias, 0, n_out), scale=1.0)
                    else:
                        res_v = pv(resid, n_out)[:, t * 8 + 1: t * 8 + 9, 2:66]
                        y1_t = tmp.tile([128, 512], F32, tag="y1")
                        nc.vector.scalar_tensor_tensor(
                            out=y1_t[0:n_out].rearrange("p (h w) -> p h w", h=8),
                            in0=ps_t[0:n_out].rearrange("p (h w) -> p h w", h=8),
                            scalar=bias_ap(c3bias, 0, n_out),
                            in1=res_v.bitcast(F32) if MM_DT == F32R else res_v,
                            op0=OP.add, op1=OP.add)
                        e_t = tmp.tile([128, 512], F32, tag="e")
                        nc.scalar.activation(
                            out=e_t[0:n_out], in_=y1_t[0:n_out], func=AF.Exp,
                            bias=bias_ap("neg1", 0, n_out), scale=1.0)
                    nc.vector.scalar_tensor_tensor(
                        out=dst_v,
                        in0=e_t[0:n_out].rearrange("p (h w) -> p h w", h=8),
                        scalar=1.0,
                        in1=y1_t[0:n_out].rearrange("p (h w) -> p h w", h=8),
                        op0=OP.min, op1=OP.max)

            for b in range(B_LOC):
                # ---- load image: x1 into padded interior (f32r), x2 to f32r
                st_x1 = stage.tile([C2, H * W], F32, tag="stage_x1")
                nc.sync.dma_start(
                    out=st_x1,
                    in_=x_d.ap()[b, 0:C2].rearrange("c h w -> c (h w)"))
                st_x2 = stage.tile([C2, H * W], F32, tag="stage_x2")
                nc.sync.dma_start(
                    out=st_x2,
                    in_=x_d.ap()[b, C2:C_IN].rearrange("c h w -> c (h w)"))
                x1_int = pv(x1pad, C2)[:, 1:65, 2:66]
                nc.vector.tensor_copy(
                    out=x1_int,
                    in_=st_x1.rearrange("p (h w) -> p h w", h=H))
                x2r = pads.tile([C2, H * W], MM_DT, tag="x2r")
                nc.vector.tensor_copy(out=x2r, in_=st_x2)

                # ---- conv net
                conv(x1pad, C2, "w_in", 9, HID, "in_e", "in_y1", dst=hbuf["hA"])
                io = [("hA", "hB"), ("hB", "hA")]
                for i in range(2):
                    sA, sB = io[i]
                    conv(hbuf[sA], HID, f"w_c1_{i}", 9, HID,
                         f"c1_{i}_e", f"c1_{i}_y1", dst=hbuf["h1"])
                    conv(hbuf["h1"], HID, f"w_c2_{i}", 1, HID,
                         f"c2_{i}_e", f"c2_{i}_y1", dst=hbuf["h2"])
                    conv(hbuf["h2"], HID, f"w_c3_{i}", 9, HID,
                         None, None, dst=hbuf[sB], resid=hbuf[sA],
                         c3bias=f"c3_{i}_e")

                # ---- out conv + coupling
                tr_ps = pst.tile([1, 512], F32, tag="tr")

                def couple(t, ps_t):
                    t_t = tmp2.tile([64, 512], MM_DT, tag="t")
                    nc.scalar.activation(
                        out=t_t, in_=ps_t[0:64], func=AF.Tanh,
                        bias=bias_ap("out", 0, 64), scale=sc["s_scale"])
                    sh_t = tmp2.tile([8, 512], F32, tag="sh")
                    nc.scalar.activation(
                        out=sh_t, in_=ps_t[64:72], func=AF.Identity,
                        bias=bias_ap("out", 64, 72), scale=1.0)
                    # trace accumulation across tiles
                    nc.tensor.matmul(tr_ps, wt["sel"], t_t,
                                     start=(t == 0), stop=(t == NT - 1))
                    # Horner series
                    x2_t = x2r[:, t * 512:(t + 1) * 512]
                    v_ps = pss.tile([64, 512], F32, tag="v")
                    nc.tensor.matmul(v_ps, wt["r"], x2_t, start=True, stop=True)
                    for k in range(NK, 1, -1):
                        p_t = tmp2.tile([64, 512], MM_DT, tag="p")
                        nc.vector.scalar_tensor_tensor(
                            out=p_t, in0=t_t.bitcast(F32) if MM_DT == F32R else t_t,
                            scalar=sc["rescale"], in1=v_ps,
                            op0=OP.mult, op1=OP.mult)
                        v2 = pss.tile([64, 512], F32, tag="v")
                        nc.tensor.matmul(v2, wt["mk"][:, (k - 2) * 64:(k - 1) * 64],
                                         p_t, start=True, stop=False)
                        nc.tensor.matmul(v2, wt["r"], x2_t,
                                         start=False, stop=True)
                        v_ps = v2
                    p_t = tmp2.tile([64, 512], MM_DT, tag="p")
                    nc.vector.scalar_tensor_tensor(
                        out=p_t, in0=t_t.bitcast(F32) if MM_DT == F32R else t_t, scalar=sc["rescale"],
                        in1=v_ps, op0=OP.mult, op1=OP.mult)
                    y_ps = pss.tile([8, 512], F32, tag="y")
                    nc.tensor.matmul(y_ps, wt["s1"], p_t, start=True, stop=False)
                    nc.tensor.matmul(y_ps, wt["i8"], x2_t, start=False, stop=True)
                    out_t = tmp.tile([8, 512], F32, tag="out")
                    nc.vector.tensor_add(out=out_t, in0=y_ps, in1=sh_t)
                    nc.sync.dma_start(
                        out=y2_d.ap()[b, :, t * 8:(t + 1) * 8, :],
                        in_=out_t.rearrange("p (h w) -> p h w", h=8))

                conv(hbuf["hA"], HID, "w_out", 9, OUT_CH, None, None,
                     out_cb=couple)
                nc.vector.tensor_reduce(
                    out=ld_sb[:, b:b + 1], in_=tr_ps,
                    axis=mybir.AxisListType.X, op=OP.add)
            nc.sync.dma_start(out=ld_d.ap(), in_=ld_sb)

    nc.finalize()
    return nc


# ------------------------------------------------------------------- driver
_CACHE = {}


def _get_program(P):
    key = "prog"
    if key not in _CACHE:
        _CACHE[key] = build_program(P)
    return _CACHE[key]


def kernel(x, y=None, params=None, _want_trace=False):
    x = _np32(x)
    P = prep_host(params)
    nc = _get_program(P)

    in_maps = []
    for c in range(N_CORES):
        m = {"x": np.ascontiguousarray(x[c * B_LOC:(c + 1) * B_LOC])}
        for n in ["w_in", "w_c1_0", "w_c2_0", "w_c3_0", "w_c1_1", "w_c2_1",
                  "w_c3_1", "w_out", "mk", "r", "s1", "i8", "sel"]:
            m[n] = np.ascontiguousarray(P[n].reshape(P[n].shape[0], -1))
        m["biases"] = P["biases"]
        in_maps.append(m)

    res = run_bass_kernel_spmd(nc, in_maps, core_ids=list(range(N_CORES)),
                               trace=_want_trace)

    y2 = np.concatenate([r["y2"] for r in res.results], axis=0)
    ld = np.concatenate([r["ld"].reshape(-1) for r in res.results], axis=0)
    ld = ld + np.float32(8 * H * W * P["scalars"]["reshift"])
    out = np.concatenate([x[:, 0:C2], y2], axis=1).astype(np.float32)
    if _want_trace:
        return (out, ld.astype(np.float32)), res
    return out, ld.astype(np.float32)
